# revision 1
# baseline (speedup 1.0000x reference)
"""CrossNetMix (moe_routing) Trainium2 Bass kernel.

Math per layer i (reference):
    g  = softmax(xl @ gate_w.T)                       # [B,E]
    t1 = tanh(einsum('erf,bf->ber', V[i], xl))        # [B,E,R]
    t2 = tanh(einsum('esr,ber->bes', C[i], t1))       # [B,E,R]
    t3 = einsum('efs,bes->bef', U[i], t2) + bias[i]   # [B,E,F]
    xl = einsum('bef,be->bf', x0*t3, g) + xl

Key identities used:
  - sum_e g[b,e]*(Uout_e[b,f] + bias[f]) = sum_e g*Uout + bias  (softmax sums to 1)
  - xl_{i+1} = x0 * (1 + sum_{j<=i} s_j)  where s_j = sum_e g*Uout + bias_j,
    so a running accumulator in PSUM (initialized to 1.0) replaces the
    explicit residual stream.

On-chip layout is feature-major ([F, B] transposed); the host pre-transposes
x and all weight matrices so the device does zero transposes.  Batch is
data-parallel across 8 cores (2048 rows each), processed in groups of 256
columns; all matmuls are float32r (TF32-like) with N=256 for full PE rate.
"""

import numpy as np

import concourse.bacc as bacc
import concourse.bass as bass
import concourse.tile as tile
from concourse import mybir
from concourse.bass_utils import run_bass_kernel_spmd

F32 = mybir.dt.float32
F32R = mybir.dt.float32r
AF = mybir.ActivationFunctionType
ALU = mybir.AluOpType

B, F, R, E, L = 16384, 1024, 64, 4, 3
N_CORES = 8
BC = B // N_CORES          # batch per core
NG = 256                   # batch-group (matmul N)
GROUPS = BC // NG          # 8
NCH = F // 128             # 8 feature chunks
NPAIR = E // 2             # 2 expert pairs


def build_program(with_bias: bool, timing_reps: int = 1):
    nc = bacc.Bacc("TRN2", target_bir_lowering=False, debug=False)

    xT = nc.dram_tensor("xT", [F, BC], F32R, kind="ExternalInput")
    vw = nc.dram_tensor("vw", [L, NPAIR, F, 128], F32R, kind="ExternalInput")
    cw = nc.dram_tensor("cw", [L, NPAIR, 128, 128], F32R, kind="ExternalInput")
    uw = nc.dram_tensor("uw", [L, NPAIR, 128, F], F32R, kind="ExternalInput")
    gw = nc.dram_tensor("gw", [F, E], F32R, kind="ExternalInput")
    p4 = nc.dram_tensor("p4", [NPAIR, E, 128], F32R, kind="ExternalInput")
    ones = nc.dram_tensor("ones", [4, 512], F32R, kind="ExternalInput")
    bcb = nc.dram_tensor("bcb", [L, F], F32, kind="ExternalInput")
    outT = nc.dram_tensor("outT", [F, BC], F32, kind="ExternalOutput")

    with tile.TileContext(nc) as tc:
        with (
            nc.allow_low_precision(
                reason="float32r (TF32-like) matmul inputs are intentional"
            ),
            tc.tile_pool(name="wpool", bufs=1) as wpool,
            tc.tile_pool(name="xpool", bufs=2) as xpool,
            tc.tile_pool(name="work", bufs=2) as work,
            tc.tile_pool(name="accpool", bufs=1, space="PSUM") as accpool,
            tc.tile_pool(name="gatepool", bufs=1, space="PSUM") as gatepool,
            tc.tile_pool(name="gbpool", bufs=1, space="PSUM") as gbpool,
            tc.tile_pool(name="t1pool", bufs=1, space="PSUM") as t1pool,
            tc.tile_pool(name="t2pool", bufs=1, space="PSUM") as t2pool,
        ):
            # ---- resident weights (loaded once) ----
            vws = wpool.tile([128, L, NPAIR, NCH, 128], F32R)
            nc.sync.dma_start(
                out=vws, in_=vw.rearrange("l q (c p) m -> p l q c m", p=128)
            )
            uws = wpool.tile([128, L, NPAIR, NCH, 128], F32R)
            nc.sync.dma_start(
                out=uws, in_=uw.rearrange("l q k (c m) -> k l q c m", m=128)
            )
            cws = wpool.tile([128, L, NPAIR, 128], F32R)
            nc.sync.dma_start(out=cws, in_=cw.rearrange("l q k m -> k l q m"))
            gws = wpool.tile([128, NCH, E], F32R)
            nc.sync.dma_start(out=gws, in_=gw.rearrange("(c p) e -> p c e", p=128))
            p4s = wpool.tile([E, NPAIR, 128], F32R)
            nc.sync.dma_start(out=p4s, in_=p4.rearrange("q e m -> e q m"))
            bcs = None
            if with_bias:
                bcs = wpool.tile([128, L, NCH], F32)
                nc.sync.dma_start(
                    out=bcs, in_=bcb.rearrange("l (c p) -> p l c", p=128)
                )
            onest = wpool.tile([4, 512], F32R)
            nc.sync.dma_start(out=onest, in_=ones.ap())
            ones_r = onest[0:1, :]
            ones4 = onest[0:4, 0:1]

            xT_r = xT.rearrange("(c p) b -> p c b", p=128)
            outT_r = outT.rearrange("(c p) b -> p c b", p=128)

            if timing_reps > 1:
                # repeat the whole computation on-device to amortize the
                # ~80 ms RPC dispatch overhead when measuring exec time
                import contextlib
                rep_loop = tc.For_i(0, timing_reps, 1)
            else:
                import contextlib
                rep_loop = contextlib.nullcontext()
            with rep_loop:
                _build_body(nc, tc, locals())

    nc.compile()
    return nc


def _build_body(nc, tc, env):
    (wpool, xpool, work, accpool, gatepool, gbpool, t1pool, t2pool) = (
        env["wpool"], env["xpool"], env["work"], env["accpool"],
        env["gatepool"], env["gbpool"], env["t1pool"], env["t2pool"],
    )
    vws, uws, cws, gws, p4s, bcs = (
        env["vws"], env["uws"], env["cws"], env["gws"], env["p4s"], env["bcs"],
    )
    ones_r, ones4, xT_r, outT_r = (
        env["ones_r"], env["ones4"], env["xT_r"], env["outT_r"],
    )
    with_bias = env["with_bias"]
    if True:
            for g in range(GROUPS):
                gsl = slice(g * NG, (g + 1) * NG)

                x0s = xpool.tile([128, NCH * NG], F32R, tag="x0s")
                nc.sync.dma_start(
                    out=x0s.rearrange("p (c b) -> p c b", c=NCH),
                    in_=xT_r[:, :, gsl],
                )
                xls = xpool.tile([128, NCH * NG], F32R, tag="xls")

                # PSUM running accumulator: 4 banks, init to 1.0
                accp = accpool.tile([128, NCH * NG], F32)
                for j in range(4):
                    nc.tensor.matmul(
                        accp[:, j * 512:(j + 1) * 512],
                        ones_r[0:1, 0:128],
                        ones_r[0:1, 0:512],
                        start=True, stop=False, skip_group_check=True,
                    )

                for l in range(L):
                    src = x0s if l == 0 else xls

                    # ---- gate logits z[e,b] and softmax ----
                    gatep = gatepool.tile([4, 512], F32)
                    z = gatep[:, 0:NG]
                    for c in range(NCH):
                        nc.tensor.matmul(
                            z, gws[:, c, :], src[:, c * NG:(c + 1) * NG],
                            start=(c == 0), stop=(c == NCH - 1),
                        )
                    ez = work.tile([4, NG], F32R, tag="ez")
                    nc.scalar.activation(out=ez, in_=z, func=AF.Exp)
                    # sum over experts -> borrow a corner of the t2 bank
                    t2p = t2pool.tile([128, 512], F32)
                    s_ = t2p[0:1, 0:NG]
                    nc.tensor.matmul(s_, ones4, ez, start=True, stop=True)
                    rs = work.tile([1, NG], F32R, tag="rs")
                    nc.vector.reciprocal(out=rs, in_=s_)
                    rb4 = gatep[:, NG:2 * NG]
                    nc.tensor.matmul(
                        rb4, ones_r[0:1, 0:4], rs,
                        start=True, stop=True, skip_group_check=True,
                    )
                    gn = work.tile([4, NG], F32R, tag="gn")
                    nc.vector.tensor_mul(gn, ez, rb4)
                    # broadcast each expert's gate over 64 partitions (pair layout)
                    gbp = gbpool.tile([128, 2 * NG], F32)
                    for q in range(NPAIR):
                        nc.tensor.matmul(
                            gbp[:, q * NG:(q + 1) * NG],
                            p4s[:, q, :], gn,
                            start=(q == 0), stop=(q == NPAIR - 1),
                        )

                    # ---- t1 = tanh(V @ xl), experts packed in pairs ----
                    t1p = t1pool.tile([128, 2 * NG], F32)
                    for q in range(NPAIR):
                        for c in range(NCH):
                            nc.tensor.matmul(
                                t1p[:, q * NG:(q + 1) * NG],
                                vws[:, l, q, c, :],
                                src[:, c * NG:(c + 1) * NG],
                                start=(q == 0 and c == 0),
                                stop=(q == NPAIR - 1 and c == NCH - 1),
                            )
                    t1s = work.tile([128, 2 * NG], F32R, tag="t1s")
                    nc.scalar.activation(out=t1s, in_=t1p, func=AF.Tanh)

                    # ---- t2 = tanh(C @ t1) * gate (block-diag pair C) ----
                    for q in range(NPAIR):
                        nc.tensor.matmul(
                            t2p[:, q * NG:(q + 1) * NG],
                            cws[:, l, q, :],
                            t1s[:, q * NG:(q + 1) * NG],
                            start=(q == 0), stop=(q == NPAIR - 1),
                        )
                    t2t = work.tile([128, 2 * NG], F32, tag="t2t")
                    nc.scalar.activation(out=t2t, in_=t2p, func=AF.Tanh)
                    t2s = work.tile([128, 2 * NG], F32R, tag="t2s")
                    nc.vector.tensor_mul(t2s, t2t, gbp)

                    # ---- U projection accumulates into the running PSUM acc ----
                    for c in range(NCH):
                        for q in range(NPAIR):
                            # stop closes the chunk's group for the sim's
                            # mid-group read check (no-op on HW); accumulation
                            # continues next layer via has_written bits
                            nc.tensor.matmul(
                                accp[:, c * NG:(c + 1) * NG],
                                uws[:, l, q, c, :],
                                t2s[:, q * NG:(q + 1) * NG],
                                start=False, stop=(q == NPAIR - 1),
                                skip_group_check=True,
                            )

                    # ---- xl_{l+1} = x0 * (acc [+ cumulative bias]) ----
                    # final layer keeps full fp32 (feeds no matmul)
                    if l == L - 1:
                        dst = work.tile([128, NCH * NG], F32, tag="outs")
                    else:
                        dst = xls
                    if with_bias:
                        for c in range(NCH):
                            csl = slice(c * NG, (c + 1) * NG)
                            nc.vector.scalar_tensor_tensor(
                                out=dst[:, csl],
                                in0=accp[:, csl],
                                scalar=bcs[:, l, c:c + 1],
                                in1=x0s[:, csl],
                                op0=ALU.add, op1=ALU.mult,
                            )
                    else:
                        nc.vector.tensor_mul(dst, accp, x0s)

                nc.sync.dma_start(
                    out=outT_r[:, :, gsl],
                    in_=dst.rearrange("p (c b) -> p c b", c=NCH),
                )


_PROG_CACHE: dict[bool, object] = {}


def _get_program(with_bias: bool):
    if with_bias not in _PROG_CACHE:
        _PROG_CACHE[with_bias] = build_program(with_bias)
    return _PROG_CACHE[with_bias]


def prepare_inputs(x, U, V, C, bias, gate_w):
    """Host-side prep: transpose/pack weights, shard x. Returns in_maps."""
    x = np.asarray(x, dtype=np.float32)
    U = np.asarray(U, dtype=np.float32)
    V = np.asarray(V, dtype=np.float32)
    C = np.asarray(C, dtype=np.float32)
    bias = np.asarray(bias, dtype=np.float32)
    gate_w = np.asarray(gate_w, dtype=np.float32)

    # V[l,e] is [R,F]; lhsT needs [F,R] chunks, experts packed in pairs -> [l,q,F,128]
    vt = V.transpose(0, 1, 3, 2)                     # [L,E,F,R]
    vw = np.stack(
        [np.concatenate([vt[:, 2 * q], vt[:, 2 * q + 1]], axis=-1)
         for q in range(NPAIR)], axis=1,
    )                                                # [L,NPAIR,F,128]

    # C[l,e] is [S,R]; lhsT needs [R,S]; block-diag per pair -> [l,q,128,128]
    ct = C.transpose(0, 1, 3, 2)                     # [L,E,R,S]
    cwm = np.zeros((L, NPAIR, 128, 128), dtype=np.float32)
    for q in range(NPAIR):
        cwm[:, q, :R, :R] = ct[:, 2 * q]
        cwm[:, q, R:, R:] = ct[:, 2 * q + 1]

    # U[l,e] is [F,S]; lhsT needs [S,F] stacked per pair -> [l,q,128,F]
    ut = U.transpose(0, 1, 3, 2)                     # [L,E,S,F]
    uwm = np.stack(
        [np.concatenate([ut[:, 2 * q], ut[:, 2 * q + 1]], axis=1)
         for q in range(NPAIR)], axis=1,
    )                                                # [L,NPAIR,128,F]

    gwt = np.ascontiguousarray(gate_w.T)             # [F,E]

    # broadcast patterns: p4[q,e,m] = 1 if expert e feeds rows of pair q
    p4m = np.zeros((NPAIR, E, 128), dtype=np.float32)
    for q in range(NPAIR):
        p4m[q, 2 * q, :R] = 1.0
        p4m[q, 2 * q + 1, R:] = 1.0

    bias2 = bias[:, :, 0] if bias.ndim == 3 else bias
    bcb = np.cumsum(bias2, axis=0).astype(np.float32)  # [L,F]
    with_bias = bool(np.any(bias2))

    xt = np.ascontiguousarray(x.T)                   # [F,B]

    shared = {
        "vw": np.ascontiguousarray(vw),
        "cw": np.ascontiguousarray(cwm),
        "uw": np.ascontiguousarray(uwm),
        "gw": gwt,
        "p4": p4m,
        "ones": np.ones((4, 512), dtype=np.float32),
        "bcb": np.ascontiguousarray(bcb),
    }
    in_maps = []
    for k in range(N_CORES):
        m = dict(shared)
        m["xT"] = np.ascontiguousarray(xt[:, k * BC:(k + 1) * BC])
        in_maps.append(m)
    return in_maps, with_bias


def run(in_maps, with_bias, **kw):
    nc = _get_program(with_bias)
    return run_bass_kernel_spmd(nc, in_maps, list(range(N_CORES)), **kw)


def make_timed_runner(in_maps, with_bias, iters=1):
    """Build a sharded jit callable with device-resident inputs for timing.

    Mirrors bass2jax.run_bass_via_pjrt's multi-core path but keeps the
    non-donated inputs on device so repeated calls measure NEFF execution
    + dispatch only."""
    import jax
    import concourse.mybir as mybir_
    from jax.experimental.shard_map import shard_map
    from jax.sharding import Mesh, PartitionSpec, NamedSharding
    from concourse import bass2jax as b2j

    b2j.install_neuronx_cc_hook()
    nc = _get_program(with_bias)

    partition_name = (
        nc.partition_id_tensor.name if nc.partition_id_tensor else None
    )
    in_names, out_names, out_avals, zero_outs = [], [], [], []
    for alloc in nc.m.functions[0].allocations:
        if not isinstance(alloc, mybir_.MemoryLocationSet):
            continue
        name = alloc.memorylocations[0].name
        if alloc.kind == "ExternalInput":
            if name != partition_name:
                in_names.append(name)
        elif alloc.kind == "ExternalOutput":
            shape = tuple(alloc.tensor_shape)
            dtype = mybir_.dt.np(alloc.dtype)
            out_names.append(name)
            out_avals.append(jax.core.ShapedArray(shape, dtype))
            zero_outs.append(np.zeros(shape, dtype))
    n_params = len(in_names)
    n_outs = len(out_avals)
    all_in_names = in_names + out_names
    if partition_name is not None:
        all_in_names = all_in_names + [partition_name]
    donate = tuple(range(n_params, n_params + n_outs))

    def _body(*args):
        ins = list(args[:n_params])
        bufs = list(args[n_params:])
        for _ in range(iters):
            operands = ins + bufs
            if partition_name is not None:
                operands.append(b2j.partition_id_tensor())
            outs = b2j._bass_exec_p.bind(
                *operands,
                out_avals=tuple(out_avals),
                in_names=tuple(all_in_names),
                out_names=tuple(out_names),
                lowering_input_output_aliases=(),
                sim_require_finite=True,
                sim_require_nnan=True,
                nc=nc,
            )
            # chain: reuse outputs as next call's scratch buffers so the
            # executions are data-dependent (serialized on device)
            bufs = list(outs)
        return tuple(bufs)

    devices = jax.devices()[:N_CORES]
    mesh = Mesh(np.asarray(devices), ("core",))
    in_specs = (PartitionSpec("core"),) * (n_params + n_outs)
    out_specs = (PartitionSpec("core"),) * n_outs
    sharded = jax.jit(
        shard_map(_body, mesh=mesh, in_specs=in_specs, out_specs=out_specs,
                  check_rep=False),
        donate_argnums=donate, keep_unused=True,
    )
    sh = NamedSharding(mesh, PartitionSpec("core"))
    concat_in = [
        np.concatenate([np.asarray(in_maps[c][nm]) for c in range(N_CORES)], axis=0)
        for nm in in_names
    ]
    ins_dev = [jax.device_put(a, sh) for a in concat_in]

    def call():
        zeros_dev = [
            jax.device_put(np.zeros((N_CORES * z.shape[0], *z.shape[1:]), z.dtype), sh)
            for z in zero_outs
        ]
        jax.block_until_ready(zeros_dev)
        import time as _t
        t0 = _t.perf_counter()
        out = sharded(*ins_dev, *zeros_dev)
        jax.block_until_ready(out)
        return _t.perf_counter() - t0, out

    return call


def kernel(x, U, V, C, bias, gate_w):
    in_maps, with_bias = prepare_inputs(x, U, V, C, bias, gate_w)
    res = run(in_maps, with_bias)
    out = np.empty((F, B), dtype=np.float32)
    for k in range(N_CORES):
        out[:, k * BC:(k + 1) * BC] = res.results[k]["outT"]
    return np.ascontiguousarray(out.T)



# revision 36
# speedup vs baseline: 228.4732x; 228.4732x over previous
"""CrossNetMix (moe_routing) Trainium2 Bass kernel.

Math per layer i (reference):
    g  = softmax(xl @ gate_w.T)                       # [B,E]
    t1 = tanh(einsum('erf,bf->ber', V[i], xl))        # [B,E,R]
    t2 = tanh(einsum('esr,ber->bes', C[i], t1))       # [B,E,R]
    t3 = einsum('efs,bes->bef', U[i], t2) + bias[i]   # [B,E,F]
    xl = einsum('bef,be->bf', x0*t3, g) + xl

Key identities used:
  - sum_e g[b,e]*(Uout_e[b,f] + bias[f]) = sum_e g*Uout + bias  (softmax sums to 1)
  - xl_{i+1} = x0 * (1 + cumbias_i + sum_{j<=i} s_j) where s_j = sum_e g*Uout_j,
    so a running accumulator in PSUM replaces the explicit residual stream;
    the "+1 (+cumbias)" is folded into the per-layer (acc + k) * x0 multiply.

On-chip layout is feature-major ([F, B] transposed); the host pre-packs x and
all weights into the exact SBUF layouts so every DMA is contiguous.  Batch is
data-parallel across 8 cores (2048 rows each), processed in groups of 256
columns; all matmuls are float32r (TF32-like) with N=256 for full PE rate.

PSUM budget (8 banks x 2KB): acc[128,2048]=4, t1/t2 shared pool (bufs=2)=2,
gate bank (z/s/rb4 packed)=1, gate-broadcast=1.  The per-group U accumulation
starts with start=True at layer 0 (no separate init matmuls), and the
(acc+1)*x0 update is chunked across both DVE and Pool engines so the tensor
engine of the *other* in-flight group keeps streaming during it.
"""

import numpy as np

import concourse.bacc as bacc
import concourse.bass as bass
import concourse.tile as tile
from concourse import mybir
from concourse.bass_utils import run_bass_kernel_spmd

F32 = mybir.dt.float32
F32R = mybir.dt.float32r
AF = mybir.ActivationFunctionType
ALU = mybir.AluOpType

B, F, R, E, L = 16384, 1024, 64, 4, 3
N_CORES = 8
BC = B // N_CORES          # batch per core
NG = 256                   # batch-group (matmul N)
GROUPS = BC // NG          # 8
NCH = F // 128             # 8 feature chunks
NPAIR = E // 2             # 2 expert pairs


def build_program(with_bias: bool):
    nc = bacc.Bacc("TRN2", target_bir_lowering=False, debug=False)

    # All inputs pre-packed on host to the exact on-chip layout (partition
    # dim first) so every DMA is a contiguous copy.
    xh = nc.dram_tensor("xh", [128, GROUPS, NCH, NG], F32R, kind="ExternalInput")
    vw = nc.dram_tensor("vw", [128, L, NPAIR, NCH, 128], F32R, kind="ExternalInput")
    cw = nc.dram_tensor("cw", [128, L, NPAIR, 128], F32R, kind="ExternalInput")
    uw = nc.dram_tensor("uw", [128, L, NPAIR, NCH, 128], F32R, kind="ExternalInput")
    gw = nc.dram_tensor("gw", [128, NCH, E], F32R, kind="ExternalInput")
    p4 = nc.dram_tensor("p4", [4, NPAIR, 128], F32R, kind="ExternalInput")
    ones = nc.dram_tensor("ones", [4, 512], F32R, kind="ExternalInput")
    bcb = nc.dram_tensor("bcb", [128, L, NCH], F32, kind="ExternalInput")
    outT = nc.dram_tensor("outT", [128, GROUPS, NCH, NG], F32, kind="ExternalOutput")

    with tile.TileContext(nc) as tc:
        with (
            nc.allow_low_precision(
                reason="float32r (TF32-like) matmul inputs are intentional"
            ),
            tc.tile_pool(name="wpool", bufs=1) as wpool,
            tc.tile_pool(name="xpool", bufs=2) as xpool,
            tc.tile_pool(name="work", bufs=2) as work,
            tc.tile_pool(name="accpool", bufs=1, space="PSUM") as accpool,
            tc.tile_pool(name="mmpool", bufs=2, space="PSUM") as mmpool,
            tc.tile_pool(name="zpool", bufs=1, space="PSUM") as zpool,
            tc.tile_pool(name="gbpool", bufs=1, space="PSUM") as gbpool,
        ):
            # ---- resident weights: issued on the Activation DMA queue,
            # split per layer, so layer-0 compute starts after ~1/3 of the
            # weight traffic while x streams on the SP queue in parallel.
            gws = wpool.tile([128, NCH, E], F32R)
            nc.scalar.dma_start(out=gws, in_=gw.ap())
            p4s = wpool.tile([4, NPAIR, 128], F32R)
            nc.scalar.dma_start(out=p4s, in_=p4.ap())
            bcs = None
            if with_bias:
                bcs = wpool.tile([128, L, NCH], F32)
                nc.scalar.dma_start(out=bcs, in_=bcb.ap())
            vws = wpool.tile([128, L, NPAIR, NCH, 128], F32R)
            cws = wpool.tile([128, L, NPAIR, 128], F32R)
            uws = wpool.tile([128, L, NPAIR, NCH, 128], F32R)
            for l in range(L):
                nc.scalar.dma_start(out=vws[:, l], in_=vw.ap()[:, l])
                nc.scalar.dma_start(out=cws[:, l], in_=cw.ap()[:, l])
                nc.scalar.dma_start(out=uws[:, l], in_=uw.ap()[:, l])

            onest = wpool.tile([4, 512], F32R)
            nc.scalar.dma_start(out=onest, in_=ones.ap())
            ones4 = onest[0:4, 0:1]   # lhsT for sum over 4 experts
            ones14 = onest[0:1, 0:4]  # lhsT for broadcast 1 -> 4 partitions
            ones_r = onest[0:1, :]

            for g in range(GROUPS):
                x0s = xpool.tile([128, NCH, NG], F32R, tag="x0s")
                nc.sync.dma_start(out=x0s, in_=xh.ap()[:, g])
                xls = xpool.tile([128, NCH, NG], F32R, tag="xls")
                outs = work.tile([128, NCH, NG], F32, tag="outs")

                # PSUM running accumulator, init to 1.0 with one bank-wide
                # matmul per bank (start=True resets a whole bank's
                # has_written bits, so sub-bank-width init is unsafe)
                accp = accpool.tile([128, NCH * NG], F32)
                for j in range(4):
                    nc.tensor.matmul(
                        accp[:, j * 512:(j + 1) * 512],
                        ones_r[0:1, 0:128],
                        ones_r[0:1, 0:512],
                        start=True, stop=False, skip_group_check=True,
                    )

                for l in range(L):
                    src = x0s if l == 0 else xls

                    # ---- gate logits z[e,b] (one PSUM bank holds z, s, rb4)
                    zb = zpool.tile([128, 2 * NG], F32)
                    z = zb[0:4, 0:NG]
                    for c in range(NCH):
                        nc.tensor.matmul(
                            z, gws[:, c, :], src[:, c],
                            start=(c == 0), stop=(c == NCH - 1),
                        )
                    ez = work.tile([4, NG], F32R, tag="ez")
                    nc.scalar.activation(out=ez, in_=z, func=AF.Exp)

                    # ---- t1 = tanh(V @ xl), experts packed in pairs ----
                    t1p = mmpool.tile([128, 2 * NG], F32, tag="mm")
                    for q in range(NPAIR):
                        for c in range(NCH):
                            nc.tensor.matmul(
                                t1p[:, q * NG:(q + 1) * NG],
                                vws[:, l, q, c, :],
                                src[:, c],
                                start=(q == 0 and c == 0),
                                stop=(q == NPAIR - 1 and c == NCH - 1),
                            )
                    t1s = work.tile([128, NPAIR, NG], F32R, tag="t1s")
                    for q in range(NPAIR):
                        nc.scalar.activation(
                            out=t1s[:, q], in_=t1p[:, q * NG:(q + 1) * NG],
                            func=AF.Tanh,
                        )

                    # ---- softmax denominator (PE sum; fast approx recip) ----
                    # s_ shares the rb4 region (partition 0 of the second
                    # half-bank): rb4 is only written after recip reads s_
                    s_ = zb[0:1, NG:2 * NG]
                    nc.tensor.matmul(
                        s_, ones4, ez, start=True, stop=True,
                        skip_group_check=True,
                    )
                    rs = work.tile([1, NG], F32, tag="rs")
                    nc.vector.reciprocal_approx_fast(out=rs, in_=s_)
                    # f32r round-trip via Act copy (matmul rhs must be f32r
                    # from an f32r-producing op to satisfy the BIR verifier)
                    rsr = work.tile([1, NG], F32R, tag="rsr")
                    nc.scalar.copy(out=rsr, in_=rs)

                    # ---- t2 = tanh(C @ t1) (block-diag pair C) ----
                    t2p = mmpool.tile([128, 2 * NG], F32, tag="mm")
                    for q in range(NPAIR):
                        nc.tensor.matmul(
                            t2p[:, q * NG:(q + 1) * NG],
                            cws[:, l, q, :],
                            t1s[:, q],
                            start=(q == 0), stop=(q == NPAIR - 1),
                        )

                    rb4 = zb[0:4, NG:2 * NG]
                    nc.tensor.matmul(
                        rb4, ones14, rsr, start=True, stop=True,
                        skip_group_check=True,
                    )
                    gn = work.tile([4, NG], F32R, tag="gn")
                    nc.vector.tensor_mul(gn, ez, rb4)
                    # broadcast each expert's gate over its 64 rows (pair layout)
                    gbp = gbpool.tile([128, 2 * NG], F32)
                    for q in range(NPAIR):
                        nc.tensor.matmul(
                            gbp[:, q * NG:(q + 1) * NG],
                            p4s[:, q, :], gn,
                            start=(q == 0), stop=(q == NPAIR - 1),
                        )

                    t2t = work.tile([128, NPAIR, NG], F32, tag="t2t")
                    for q in range(NPAIR):
                        nc.scalar.activation(
                            out=t2t[:, q], in_=t2p[:, q * NG:(q + 1) * NG],
                            func=AF.Tanh,
                        )
                    t2s = work.tile([128, NPAIR, NG], F32R, tag="t2s")
                    nc.vector.tensor_mul(t2s[:, 0], t2t[:, 0], gbp[:, 0:NG])
                    nc.vector.tensor_mul(t2s[:, 1], t2t[:, 1], gbp[:, NG:2 * NG])

                    # ---- U projection accumulates into the running PSUM acc
                    for c in range(NCH):
                        for q in range(NPAIR):
                            # stop closes the chunk's group for the sim's
                            # mid-group read check (no-op on HW); accumulation
                            # continues next layer via has_written bits
                            nc.tensor.matmul(
                                accp[:, c * NG:(c + 1) * NG],
                                uws[:, l, q, c, :],
                                t2s[:, q],
                                start=False,
                                stop=(q == NPAIR - 1),
                                skip_group_check=True,
                            )

                    # ---- xl_{l+1} = x0 * (acc [+ cumulative bias]) ----
                    # (GPSIMD/Pool cannot read PSUM, so these are all DVE;
                    # chunked so downstream matmuls unblock per chunk)
                    dst = outs if l == L - 1 else xls
                    if with_bias:
                        for c in range(NCH):
                            nc.vector.scalar_tensor_tensor(
                                out=dst[:, c],
                                in0=accp[:, c * NG:(c + 1) * NG],
                                scalar=bcs[:, l, c:c + 1],
                                in1=x0s[:, c],
                                op0=ALU.add, op1=ALU.mult,
                            )
                    else:
                        for c0 in (0, 2, 4, 6):
                            nc.vector.tensor_mul(
                                dst[:, c0:c0 + 2],
                                accp[:, c0 * NG:(c0 + 2) * NG],
                                x0s[:, c0:c0 + 2],
                            )

                nc.sync.dma_start(out=outT.ap()[:, g], in_=outs)

    nc.compile()
    return nc


_PROG_CACHE: dict[bool, object] = {}


def _get_program(with_bias: bool):
    if with_bias not in _PROG_CACHE:
        _PROG_CACHE[with_bias] = build_program(with_bias)
    return _PROG_CACHE[with_bias]


def prepare_inputs(x, U, V, C, bias, gate_w):
    """Host-side prep: pack weights into SBUF layouts, shard x. Returns in_maps."""
    x = np.asarray(x, dtype=np.float32)
    U = np.asarray(U, dtype=np.float32)
    V = np.asarray(V, dtype=np.float32)
    C = np.asarray(C, dtype=np.float32)
    bias = np.asarray(bias, dtype=np.float32)
    gate_w = np.asarray(gate_w, dtype=np.float32)

    # V[l,e] is [R,F]; lhsT needs [F,R] chunks, experts packed in pairs.
    # Final layout [p, l, q, c, m]: element (l, q, f=c*128+p, m).
    vt = V.transpose(0, 1, 3, 2)                     # [L,E,F,R]
    vwm = np.stack(
        [np.concatenate([vt[:, 2 * q], vt[:, 2 * q + 1]], axis=-1)
         for q in range(NPAIR)], axis=1,
    )                                                # [L,NPAIR,F,128]
    vwh = np.ascontiguousarray(
        vwm.reshape(L, NPAIR, NCH, 128, 128).transpose(3, 0, 1, 2, 4)
    )                                                # [128,L,NPAIR,NCH,128]

    # C[l,e] is [S,R]; lhsT needs [R,S]; block-diag per pair.
    ct = C.transpose(0, 1, 3, 2)                     # [L,E,R,S]
    cwm = np.zeros((L, NPAIR, 128, 128), dtype=np.float32)
    for q in range(NPAIR):
        cwm[:, q, :R, :R] = ct[:, 2 * q]
        cwm[:, q, R:, R:] = ct[:, 2 * q + 1]
    cwh = np.ascontiguousarray(cwm.transpose(2, 0, 1, 3))   # [128,L,NPAIR,128]

    # U[l,e] is [F,S]; lhsT needs [S,F] stacked per pair.
    ut = U.transpose(0, 1, 3, 2)                     # [L,E,S,F]
    uwm = np.stack(
        [np.concatenate([ut[:, 2 * q], ut[:, 2 * q + 1]], axis=1)
         for q in range(NPAIR)], axis=1,
    )                                                # [L,NPAIR,128,F]
    uwh = np.ascontiguousarray(
        uwm.reshape(L, NPAIR, 128, NCH, 128).transpose(2, 0, 1, 3, 4)
    )                                                # [128,L,NPAIR,NCH,128]

    gwh = np.ascontiguousarray(
        gate_w.T.reshape(NCH, 128, E).transpose(1, 0, 2)
    )                                                # [128,NCH,E]

    # broadcast patterns: p4[e,q,m] = 1 if expert e feeds rows m of pair q
    p4h = np.zeros((4, NPAIR, 128), dtype=np.float32)
    for q in range(NPAIR):
        p4h[2 * q, q, :R] = 1.0
        p4h[2 * q + 1, q, R:] = 1.0

    bias2 = bias[:, :, 0] if bias.ndim == 3 else bias
    with_bias = bool(np.any(bias2))
    bcb1 = np.cumsum(bias2, axis=0)                  # [L,F] (acc starts at 1.0)
    bch = np.ascontiguousarray(
        bcb1.reshape(L, NCH, 128).transpose(2, 0, 1).astype(np.float32)
    )                                                # [128,L,NCH]

    shared = {
        "vw": vwh, "cw": cwh, "uw": uwh, "gw": gwh, "p4": p4h, "bcb": bch,
        "ones": np.ones((4, 512), dtype=np.float32),
    }
    in_maps = []
    for k in range(N_CORES):
        xc = np.ascontiguousarray(x[k * BC:(k + 1) * BC].T)  # [F,BC]
        xck = xc.reshape(NCH, 128, GROUPS, NG).transpose(1, 2, 0, 3)
        m = dict(shared)
        m["xh"] = np.ascontiguousarray(xck)          # [128,GROUPS,NCH,NG]
        in_maps.append(m)
    return in_maps, with_bias


def unpack_output(res) -> np.ndarray:
    """Assemble the [B,F] float32 output from per-core outT tensors."""
    out = np.empty((B, F), dtype=np.float32)
    for k in range(N_CORES):
        o = res.results[k]["outT"]                   # [128,GROUPS,NCH,NG]
        oc = o.transpose(2, 0, 1, 3).reshape(F, BC)  # [F,BC]
        out[k * BC:(k + 1) * BC] = oc.T
    return out


def run(in_maps, with_bias, **kw):
    nc = _get_program(with_bias)
    return run_bass_kernel_spmd(nc, in_maps, list(range(N_CORES)), **kw)


def kernel(x, U, V, C, bias, gate_w):
    in_maps, with_bias = prepare_inputs(x, U, V, C, bias, gate_w)
    res = run(in_maps, with_bias)
    return unpack_output(res)


# revision 38
# speedup vs baseline: 282.6912x; 1.2373x over previous
"""CrossNetMix (moe_routing) Trainium2 Bass kernel.

Math per layer i (reference):
    g  = softmax(xl @ gate_w.T)                       # [B,E]
    t1 = tanh(einsum('erf,bf->ber', V[i], xl))        # [B,E,R]
    t2 = tanh(einsum('esr,ber->bes', C[i], t1))       # [B,E,R]
    t3 = einsum('efs,bes->bef', U[i], t2) + bias[i]   # [B,E,F]
    xl = einsum('bef,be->bf', x0*t3, g) + xl

Key identities used:
  - sum_e g[b,e]*(Uout_e[b,f] + bias[f]) = sum_e g*Uout + bias  (softmax sums to 1)
  - xl_{i+1} = x0 * a_i with a_i = a_{i-1} + s_i + bias_i, a_{-1} = 1, where
    s_i = sum_e g*Uout_i — the multiplier state `a` lives in SBUF and the
    per-layer U projection goes to a transient PSUM tile.

Layout: feature-major on chip ([F, B] transposed); the host pre-packs x and
all weights into the exact SBUF layouts so every DMA is contiguous.  Batch is
data-parallel across 8 cores (2048 rows each), processed in groups of 256
columns; all matmuls are float32r (TF32-like) with N=256 for full PE rate.

Schedule: the tensor engine only reaches its top clock after ~3us of
*uninterrupted* execution (p-state ramp), so the loop nest is layer-major
over blocks of 4 batch groups — between a group's layer end and its next
layer sit three other groups' matmuls (~18us of PE work), which lets the
tile list-scheduler hide every cross-engine chain (softmax, tanh, a-update)
without the PE ever waiting.  Engine split per layer-group: PE 46 matmuls,
Act exp+4x tanh+copy, DVE recip/gate scaling/a+=upsum (PSUM reads),
Pool xl=x0*a (SBUF only — Pool cannot access PSUM).

PSUM (8 banks): upsum[128,2048]=4, t1/t2 shared rotating pool=2, gate bank
(z/s/rb4 packed)=1, gate-broadcast=1.  U matmuls open each upsum bank with
start=True on the bank's first chunk only (start resets the whole bank's
has_written bits); all 16 become ready simultaneously (same rhs), so the
scheduler keeps their emission order and the opener lands first.
"""

import numpy as np

import concourse.bacc as bacc
import concourse.bass as bass
import concourse.tile as tile
from concourse import mybir
from concourse.bass_utils import run_bass_kernel_spmd

F32 = mybir.dt.float32
F32R = mybir.dt.float32r
AF = mybir.ActivationFunctionType
ALU = mybir.AluOpType

B, F, R, E, L = 16384, 1024, 64, 4, 3
N_CORES = 8
BC = B // N_CORES          # batch per core
NG = 256                   # batch-group (matmul N)
GROUPS = BC // NG          # 8
NCH = F // 128             # 8 feature chunks
NPAIR = E // 2             # 2 expert pairs
BLK = 4                    # groups per layer-major block
NBLK = GROUPS // BLK


def build_program(with_bias: bool):
    nc = bacc.Bacc("TRN2", target_bir_lowering=False, debug=False)

    # All inputs pre-packed on host to the exact on-chip layout (partition
    # dim first) so every DMA is a contiguous copy.
    xh = nc.dram_tensor("xh", [128, GROUPS, NCH, NG], F32R, kind="ExternalInput")
    vw = nc.dram_tensor("vw", [128, L, NPAIR, NCH, 128], F32R, kind="ExternalInput")
    cw = nc.dram_tensor("cw", [128, L, NPAIR, 128], F32R, kind="ExternalInput")
    uw = nc.dram_tensor("uw", [128, L, NPAIR, NCH, 128], F32R, kind="ExternalInput")
    gw = nc.dram_tensor("gw", [128, NCH, E], F32R, kind="ExternalInput")
    p4 = nc.dram_tensor("p4", [4, NPAIR, 128], F32R, kind="ExternalInput")
    ones = nc.dram_tensor("ones", [4, 512], F32R, kind="ExternalInput")
    bcb = nc.dram_tensor("bcb", [128, L, NCH], F32, kind="ExternalInput")
    outT = nc.dram_tensor("outT", [128, GROUPS, NCH, NG], F32, kind="ExternalOutput")

    with tile.TileContext(nc) as tc:
        with (
            nc.allow_low_precision(
                reason="float32r (TF32-like) matmul inputs are intentional"
            ),
            tc.tile_pool(name="wpool", bufs=1) as wpool,
            tc.tile_pool(name="xpool", bufs=1) as xpool,
            tc.tile_pool(name="work", bufs=2) as work,
            tc.tile_pool(name="uppool", bufs=1, space="PSUM") as uppool,
            tc.tile_pool(name="mmpool", bufs=2, space="PSUM") as mmpool,
            tc.tile_pool(name="zpool", bufs=1, space="PSUM") as zpool,
            tc.tile_pool(name="gbpool", bufs=1, space="PSUM") as gbpool,
        ):
            # ---- resident weights: issued on the Activation DMA queue,
            # split per layer, so layer-0 compute starts after ~1/3 of the
            # weight traffic while x streams on the SP queue in parallel.
            gws = wpool.tile([128, NCH, E], F32R)
            nc.scalar.dma_start(out=gws, in_=gw.ap())
            p4s = wpool.tile([4, NPAIR, 128], F32R)
            nc.scalar.dma_start(out=p4s, in_=p4.ap())
            onest = wpool.tile([4, 512], F32R)
            nc.scalar.dma_start(out=onest, in_=ones.ap())
            ones4 = onest[0:4, 0:1]   # lhsT for sum over 4 experts
            ones14 = onest[0:1, 0:4]  # lhsT for broadcast 1 -> 4 partitions
            bcs = None
            if with_bias:
                bcs = wpool.tile([128, L, NCH], F32)
                nc.scalar.dma_start(out=bcs, in_=bcb.ap())
            vws = wpool.tile([128, L, NPAIR, NCH, 128], F32R)
            cws = wpool.tile([128, L, NPAIR, 128], F32R)
            uws = wpool.tile([128, L, NPAIR, NCH, 128], F32R)
            for l in range(L):
                nc.scalar.dma_start(out=vws[:, l], in_=vw.ap()[:, l])
                nc.scalar.dma_start(out=cws[:, l], in_=cw.ap()[:, l])
                nc.scalar.dma_start(out=uws[:, l], in_=uw.ap()[:, l])

            for blk in range(NBLK):
                xs, as_, xls = [], [], []
                for gi in range(BLK):
                    x0s = xpool.tile([128, NCH, NG], F32R, tag="x0", bufs=5)
                    nc.sync.dma_start(out=x0s, in_=xh.ap()[:, blk * BLK + gi])
                    xs.append(x0s)
                    as_.append(xpool.tile([128, NCH, NG], F32, tag="a",
                                          bufs=4, name=f"a{gi}"))
                    xls.append(xpool.tile([128, NCH, NG], F32R, tag="xl",
                                          bufs=4, name=f"xl{gi}"))

                for l in range(L):
                    for gi in range(BLK):
                        g = blk * BLK + gi
                        x0s, a, xl = xs[gi], as_[gi], xls[gi]
                        src = x0s if l == 0 else xl

                        # ---- gate logits z[e,b] (bank holds z, s_, rb4) ----
                        zb = zpool.tile([128, 2 * NG], F32)
                        z = zb[0:4, 0:NG]
                        for c in range(NCH):
                            nc.tensor.matmul(
                                z, gws[:, c, :], src[:, c],
                                start=(c == 0), stop=(c == NCH - 1),
                            )
                        ez = work.tile([4, NG], F32R, tag="ez")
                        nc.scalar.activation(out=ez, in_=z, func=AF.Exp)

                        # ---- t1 = tanh(V @ xl), experts packed in pairs ----
                        t1p = mmpool.tile([128, 2 * NG], F32, tag="mm")
                        for q in range(NPAIR):
                            for c in range(NCH):
                                nc.tensor.matmul(
                                    t1p[:, q * NG:(q + 1) * NG],
                                    vws[:, l, q, c, :],
                                    src[:, c],
                                    start=(q == 0 and c == 0),
                                    stop=(q == NPAIR - 1 and c == NCH - 1),
                                )
                        t1s = work.tile([128, NPAIR, NG], F32R, tag="t1s")
                        for q in range(NPAIR):
                            nc.scalar.activation(
                                out=t1s[:, q], in_=t1p[:, q * NG:(q + 1) * NG],
                                func=AF.Tanh,
                            )

                        # ---- softmax denominator; s_ shares the rb4 region
                        # (rb4 is written only after recip consumed s_) ----
                        s_ = zb[0:1, NG:2 * NG]
                        nc.tensor.matmul(
                            s_, ones4, ez, start=True, stop=True,
                            skip_group_check=True,
                        )
                        rs = work.tile([1, NG], F32, tag="rs")
                        nc.vector.reciprocal_approx_fast(out=rs, in_=s_)
                        # f32r round-trip via Act copy (matmul rhs must come
                        # from an f32r-producing op for the BIR verifier)
                        rsr = work.tile([1, NG], F32R, tag="rsr")
                        nc.scalar.copy(out=rsr, in_=rs)

                        # ---- t2 = tanh(C @ t1) (block-diag pair C) ----
                        t2p = mmpool.tile([128, 2 * NG], F32, tag="mm")
                        for q in range(NPAIR):
                            nc.tensor.matmul(
                                t2p[:, q * NG:(q + 1) * NG],
                                cws[:, l, q, :],
                                t1s[:, q],
                                start=(q == 0), stop=(q == NPAIR - 1),
                            )

                        rb4 = zb[0:4, NG:2 * NG]
                        nc.tensor.matmul(
                            rb4, ones14, rsr, start=True, stop=True,
                            skip_group_check=True,
                        )
                        gn = work.tile([4, NG], F32R, tag="gn")
                        nc.vector.tensor_mul(gn, ez, rb4)
                        # broadcast each expert's gate over its 64 rows
                        gbp = gbpool.tile([128, 2 * NG], F32)
                        for q in range(NPAIR):
                            nc.tensor.matmul(
                                gbp[:, q * NG:(q + 1) * NG],
                                p4s[:, q, :], gn,
                                start=(q == 0), stop=(q == NPAIR - 1),
                            )

                        t2t = work.tile([128, NPAIR, NG], F32, tag="t2t")
                        for q in range(NPAIR):
                            nc.scalar.activation(
                                out=t2t[:, q], in_=t2p[:, q * NG:(q + 1) * NG],
                                func=AF.Tanh,
                            )
                        t2s = work.tile([128, NPAIR, NG], F32R, tag="t2s")
                        nc.vector.tensor_mul(t2s[:, 0], t2t[:, 0], gbp[:, 0:NG])
                        nc.vector.tensor_mul(t2s[:, 1], t2t[:, 1], gbp[:, NG:2 * NG])

                        # ---- U projection into transient PSUM ----
                        # start=True only on each bank's first chunk (c even):
                        # start resets the whole bank's has_written bits, and
                        # the odd chunk then overwrites-where-clear.
                        upsum = uppool.tile([128, NCH * NG], F32, tag="up")
                        for c in range(NCH):
                            for q in range(NPAIR):
                                nc.tensor.matmul(
                                    upsum[:, c * NG:(c + 1) * NG],
                                    uws[:, l, q, c, :],
                                    t2s[:, q],
                                    start=(q == 0 and c % 2 == 0),
                                    stop=(q == NPAIR - 1),
                                    skip_group_check=True,
                                )

                        # ---- a_l = a_{l-1} + upsum (+bias_l); a_{-1} = 1 ----
                        # (DVE: Pool cannot read PSUM)
                        if with_bias:
                            for c in range(NCH):
                                if l == 0:
                                    nc.vector.tensor_scalar_add(
                                        a[:, c], upsum[:, c * NG:(c + 1) * NG],
                                        bcs[:, 0, c:c + 1],
                                    )
                                else:
                                    nc.vector.scalar_tensor_tensor(
                                        out=a[:, c],
                                        in0=upsum[:, c * NG:(c + 1) * NG],
                                        scalar=bcs[:, l, c:c + 1],
                                        in1=a[:, c],
                                        op0=ALU.add, op1=ALU.add,
                                    )
                        else:
                            for c0 in (0, 4):
                                up_sl = upsum[:, c0 * NG:(c0 + 4) * NG]
                                if l == 0:
                                    nc.vector.tensor_scalar_add(
                                        a[:, c0:c0 + 4], up_sl, 1.0,
                                    )
                                else:
                                    nc.vector.tensor_add(
                                        a[:, c0:c0 + 4], up_sl, a[:, c0:c0 + 4],
                                    )

                        # ---- xl_{l+1} = x0 * a  (Pool, SBUF only) ----
                        if l == L - 1:
                            dst = work.tile([128, NCH, NG], F32, tag="outs")
                        else:
                            dst = xl
                        for c0 in (0, 4):
                            nc.gpsimd.tensor_mul(
                                dst[:, c0:c0 + 4], x0s[:, c0:c0 + 4],
                                a[:, c0:c0 + 4],
                            )
                        if l == L - 1:
                            nc.sync.dma_start(out=outT.ap()[:, g], in_=dst)

    nc.compile()
    return nc


_PROG_CACHE: dict[bool, object] = {}


def _get_program(with_bias: bool):
    if with_bias not in _PROG_CACHE:
        _PROG_CACHE[with_bias] = build_program(with_bias)
    return _PROG_CACHE[with_bias]


def prepare_inputs(x, U, V, C, bias, gate_w):
    """Host-side prep: pack weights into SBUF layouts, shard x. Returns in_maps."""
    x = np.asarray(x, dtype=np.float32)
    U = np.asarray(U, dtype=np.float32)
    V = np.asarray(V, dtype=np.float32)
    C = np.asarray(C, dtype=np.float32)
    bias = np.asarray(bias, dtype=np.float32)
    gate_w = np.asarray(gate_w, dtype=np.float32)

    # V[l,e] is [R,F]; lhsT needs [F,R] chunks, experts packed in pairs.
    # Final layout [p, l, q, c, m]: element (l, q, f=c*128+p, m).
    vt = V.transpose(0, 1, 3, 2)                     # [L,E,F,R]
    vwm = np.stack(
        [np.concatenate([vt[:, 2 * q], vt[:, 2 * q + 1]], axis=-1)
         for q in range(NPAIR)], axis=1,
    )                                                # [L,NPAIR,F,128]
    vwh = np.ascontiguousarray(
        vwm.reshape(L, NPAIR, NCH, 128, 128).transpose(3, 0, 1, 2, 4)
    )                                                # [128,L,NPAIR,NCH,128]

    # C[l,e] is [S,R]; lhsT needs [R,S]; block-diag per pair.
    ct = C.transpose(0, 1, 3, 2)                     # [L,E,R,S]
    cwm = np.zeros((L, NPAIR, 128, 128), dtype=np.float32)
    for q in range(NPAIR):
        cwm[:, q, :R, :R] = ct[:, 2 * q]
        cwm[:, q, R:, R:] = ct[:, 2 * q + 1]
    cwh = np.ascontiguousarray(cwm.transpose(2, 0, 1, 3))   # [128,L,NPAIR,128]

    # U[l,e] is [F,S]; lhsT needs [S,F] stacked per pair.
    ut = U.transpose(0, 1, 3, 2)                     # [L,E,S,F]
    uwm = np.stack(
        [np.concatenate([ut[:, 2 * q], ut[:, 2 * q + 1]], axis=1)
         for q in range(NPAIR)], axis=1,
    )                                                # [L,NPAIR,128,F]
    uwh = np.ascontiguousarray(
        uwm.reshape(L, NPAIR, 128, NCH, 128).transpose(2, 0, 1, 3, 4)
    )                                                # [128,L,NPAIR,NCH,128]

    gwh = np.ascontiguousarray(
        gate_w.T.reshape(NCH, 128, E).transpose(1, 0, 2)
    )                                                # [128,NCH,E]

    # broadcast patterns: p4[e,q,m] = 1 if expert e feeds rows m of pair q
    p4h = np.zeros((4, NPAIR, 128), dtype=np.float32)
    for q in range(NPAIR):
        p4h[2 * q, q, :R] = 1.0
        p4h[2 * q + 1, q, R:] = 1.0

    bias2 = bias[:, :, 0] if bias.ndim == 3 else bias
    with_bias = bool(np.any(bias2))
    # per-layer increments for the running multiplier state a (+1 at l=0)
    binc = bias2.copy()
    binc[0] += 1.0
    bch = np.ascontiguousarray(
        binc.reshape(L, NCH, 128).transpose(2, 0, 1).astype(np.float32)
    )                                                # [128,L,NCH]

    shared = {
        "vw": vwh, "cw": cwh, "uw": uwh, "gw": gwh, "p4": p4h, "bcb": bch,
        "ones": np.ones((4, 512), dtype=np.float32),
    }
    in_maps = []
    for k in range(N_CORES):
        xc = np.ascontiguousarray(x[k * BC:(k + 1) * BC].T)  # [F,BC]
        xck = xc.reshape(NCH, 128, GROUPS, NG).transpose(1, 2, 0, 3)
        m = dict(shared)
        m["xh"] = np.ascontiguousarray(xck)          # [128,GROUPS,NCH,NG]
        in_maps.append(m)
    return in_maps, with_bias


def unpack_output(res) -> np.ndarray:
    """Assemble the [B,F] float32 output from per-core outT tensors."""
    out = np.empty((B, F), dtype=np.float32)
    for k in range(N_CORES):
        o = res.results[k]["outT"]                   # [128,GROUPS,NCH,NG]
        oc = o.transpose(2, 0, 1, 3).reshape(F, BC)  # [F,BC]
        out[k * BC:(k + 1) * BC] = oc.T
    return out


def run(in_maps, with_bias, **kw):
    nc = _get_program(with_bias)
    return run_bass_kernel_spmd(nc, in_maps, list(range(N_CORES)), **kw)


def kernel(x, U, V, C, bias, gate_w):
    in_maps, with_bias = prepare_inputs(x, U, V, C, bias, gate_w)
    res = run(in_maps, with_bias)
    return unpack_output(res)


# revision 43
# speedup vs baseline: 285.6452x; 1.0104x over previous
"""CrossNetMix (moe_routing) Trainium2 Bass kernel.

Math per layer i (reference):
    g  = softmax(xl @ gate_w.T)                       # [B,E]
    t1 = tanh(einsum('erf,bf->ber', V[i], xl))        # [B,E,R]
    t2 = tanh(einsum('esr,ber->bes', C[i], t1))       # [B,E,R]
    t3 = einsum('efs,bes->bef', U[i], t2) + bias[i]   # [B,E,F]
    xl = einsum('bef,be->bf', x0*t3, g) + xl

Key identities used:
  - sum_e g[b,e]*(Uout_e[b,f] + bias[f]) = sum_e g*Uout + bias  (softmax sums to 1)
  - xl_{i+1} = x0 * a_i with a_i = a_{i-1} + s_i + bias_i, a_{-1} = 1, where
    s_i = sum_e g*Uout_i — the multiplier state `a` lives in SBUF and the
    per-layer U projection goes to a transient PSUM tile.

Layout: feature-major on chip ([F, B] transposed); the host pre-packs x and
all weights into the exact SBUF layouts so every DMA is contiguous.  Batch is
data-parallel across 8 cores (2048 rows each), processed in groups of 256
columns; all matmuls are float32r (TF32-like) with N=256 for full PE rate.

Schedule: the tensor engine only reaches its top clock after ~3us of
*uninterrupted* execution (p-state ramp), so the loop nest is layer-major
over blocks of 4 batch groups — between a group's layer end and its next
layer sit three other groups' matmuls (~18us of PE work), which lets the
tile list-scheduler hide every cross-engine chain (softmax, tanh, a-update)
without the PE ever waiting.  Engine split per layer-group: PE 46 matmuls,
Act exp+4x tanh+copy, DVE recip/gate scaling/a+=upsum (PSUM reads),
Pool xl=x0*a (SBUF only — Pool cannot access PSUM).

PSUM (8 banks): upsum[128,2048]=4, t1/t2 shared rotating pool=2, gate bank
(z/s/rb4 packed)=1, gate-broadcast=1.  U matmuls open each upsum bank with
start=True on the bank's first chunk only (start resets the whole bank's
has_written bits); all 16 become ready simultaneously (same rhs), so the
scheduler keeps their emission order and the opener lands first.
"""

import numpy as np

import concourse.bacc as bacc
import concourse.bass as bass
import concourse.tile as tile
from concourse import mybir
from concourse.bass_utils import run_bass_kernel_spmd

F32 = mybir.dt.float32
F32R = mybir.dt.float32r
AF = mybir.ActivationFunctionType
ALU = mybir.AluOpType

B, F, R, E, L = 16384, 1024, 64, 4, 3
N_CORES = 8
BC = B // N_CORES          # batch per core
NG = 256                   # batch-group (matmul N)
GROUPS = BC // NG          # 8
NCH = F // 128             # 8 feature chunks
NPAIR = E // 2             # 2 expert pairs
BLK = 4                    # groups per layer-major block
NBLK = GROUPS // BLK


def build_program(with_bias: bool):
    nc = bacc.Bacc("TRN2", target_bir_lowering=False, debug=False)

    # All inputs pre-packed on host to the exact on-chip layout (partition
    # dim first) so every DMA is a contiguous copy.
    xh = nc.dram_tensor("xh", [128, GROUPS, NCH, NG], F32R, kind="ExternalInput")
    vw = nc.dram_tensor("vw", [128, L, NPAIR, NCH, 128], F32R, kind="ExternalInput")
    cw = nc.dram_tensor("cw", [128, L, NPAIR, 128], F32R, kind="ExternalInput")
    uw = nc.dram_tensor("uw", [128, L, NPAIR, NCH, 128], F32R, kind="ExternalInput")
    gw = nc.dram_tensor("gw", [128, NCH, E], F32R, kind="ExternalInput")
    p4 = nc.dram_tensor("p4", [4, NPAIR, 128], F32R, kind="ExternalInput")
    ones = nc.dram_tensor("ones", [4, 512], F32R, kind="ExternalInput")
    bcb = nc.dram_tensor("bcb", [128, L, NCH], F32, kind="ExternalInput")
    # f32r == fp32 bit layout; declared f32r so the xl accumulate DMA and the
    # final store share dtypes (host reads the bits back as float32)
    outT = nc.dram_tensor("outT", [128, GROUPS, NCH, NG], F32R, kind="ExternalOutput")

    with tile.TileContext(nc) as tc:
        with (
            nc.allow_low_precision(
                reason="float32r (TF32-like) matmul inputs are intentional"
            ),
            tc.tile_pool(name="wpool", bufs=1) as wpool,
            tc.tile_pool(name="xpool", bufs=1) as xpool,
            tc.tile_pool(name="work", bufs=2) as work,
            tc.tile_pool(name="uppool", bufs=1, space="PSUM") as uppool,
            tc.tile_pool(name="mmpool", bufs=2, space="PSUM") as mmpool,
            tc.tile_pool(name="zpool", bufs=1, space="PSUM") as zpool,
            tc.tile_pool(name="gbpool", bufs=1, space="PSUM") as gbpool,
        ):
            # ---- resident weights: issued on the Activation DMA queue,
            # split per layer, so layer-0 compute starts after ~1/3 of the
            # weight traffic while x streams on the SP queue in parallel.
            gws = wpool.tile([128, NCH, E], F32R)
            nc.scalar.dma_start(out=gws, in_=gw.ap())
            p4s = wpool.tile([4, NPAIR, 128], F32R)
            nc.scalar.dma_start(out=p4s, in_=p4.ap())
            onest = wpool.tile([4, 512], F32R)
            nc.scalar.dma_start(out=onest, in_=ones.ap())
            ones4 = onest[0:4, 0:1]   # lhsT for sum over 4 experts
            ones14 = onest[0:1, 0:4]  # lhsT for broadcast 1 -> 4 partitions
            bcs = None
            if with_bias:
                bcs = wpool.tile([128, L, NCH], F32)
                nc.scalar.dma_start(out=bcs, in_=bcb.ap())
            vws = wpool.tile([128, L, NPAIR, NCH, 128], F32R)
            cws = wpool.tile([128, L, NPAIR, 128], F32R)
            uws = wpool.tile([128, L, NPAIR, NCH, 128], F32R)
            for l in range(L):
                nc.scalar.dma_start(out=vws[:, l], in_=vw.ap()[:, l])
                nc.scalar.dma_start(out=cws[:, l], in_=cw.ap()[:, l])
                nc.scalar.dma_start(out=uws[:, l], in_=uw.ap()[:, l])

            for blk in range(NBLK):
                xs, xls = [], []
                for gi in range(BLK):
                    x0s = xpool.tile([128, NCH, NG], F32R, tag="x0", bufs=5)
                    nc.sync.dma_start(out=x0s, in_=xh.ap()[:, blk * BLK + gi])
                    xs.append(x0s)
                    xls.append(xpool.tile([128, NCH, NG], F32R, tag="xl",
                                          bufs=4, name=f"xl{gi}"))

                for l in range(L):
                    for gi in range(BLK):
                        g = blk * BLK + gi
                        x0s, xl = xs[gi], xls[gi]
                        src = x0s if l == 0 else xl

                        # ---- gate logits z[e,b] (bank holds z, s_, rb4) ----
                        zb = zpool.tile([128, 2 * NG], F32)
                        z = zb[0:4, 0:NG]
                        for c in range(NCH):
                            nc.tensor.matmul(
                                z, gws[:, c, :], src[:, c],
                                start=(c == 0), stop=(c == NCH - 1),
                            )
                        ez = work.tile([4, NG], F32R, tag="ez")
                        nc.scalar.activation(out=ez, in_=z, func=AF.Exp)

                        # ---- t1 = tanh(V @ xl), experts packed in pairs ----
                        t1p = mmpool.tile([128, 2 * NG], F32, tag="mm")
                        for q in range(NPAIR):
                            for c in range(NCH):
                                nc.tensor.matmul(
                                    t1p[:, q * NG:(q + 1) * NG],
                                    vws[:, l, q, c, :],
                                    src[:, c],
                                    start=(q == 0 and c == 0),
                                    stop=(q == NPAIR - 1 and c == NCH - 1),
                                )
                        t1s = work.tile([128, NPAIR, NG], F32R, tag="t1s")
                        for q in range(NPAIR):
                            nc.scalar.activation(
                                out=t1s[:, q], in_=t1p[:, q * NG:(q + 1) * NG],
                                func=AF.Tanh,
                            )

                        # ---- softmax denominator; s_ shares the rb4 region
                        # (rb4 is written only after recip consumed s_) ----
                        s_ = zb[0:1, NG:2 * NG]
                        nc.tensor.matmul(
                            s_, ones4, ez, start=True, stop=True,
                            skip_group_check=True,
                        )
                        rs = work.tile([1, NG], F32, tag="rs")
                        nc.vector.reciprocal_approx_fast(out=rs, in_=s_)
                        # f32r round-trip via Act copy (matmul rhs must come
                        # from an f32r-producing op for the BIR verifier)
                        rsr = work.tile([1, NG], F32R, tag="rsr")
                        nc.scalar.copy(out=rsr, in_=rs)

                        # ---- t2 = tanh(C @ t1) (block-diag pair C) ----
                        t2p = mmpool.tile([128, 2 * NG], F32, tag="mm")
                        for q in range(NPAIR):
                            nc.tensor.matmul(
                                t2p[:, q * NG:(q + 1) * NG],
                                cws[:, l, q, :],
                                t1s[:, q],
                                start=(q == 0), stop=(q == NPAIR - 1),
                            )

                        rb4 = zb[0:4, NG:2 * NG]
                        nc.tensor.matmul(
                            rb4, ones14, rsr, start=True, stop=True,
                            skip_group_check=True,
                        )
                        gn = work.tile([4, NG], F32R, tag="gn")
                        nc.vector.tensor_mul(gn, ez, rb4)
                        # broadcast each expert's gate over its 64 rows
                        gbp = gbpool.tile([128, 2 * NG], F32)
                        for q in range(NPAIR):
                            nc.tensor.matmul(
                                gbp[:, q * NG:(q + 1) * NG],
                                p4s[:, q, :], gn,
                                start=(q == 0), stop=(q == NPAIR - 1),
                            )

                        t2t = work.tile([128, NPAIR, NG], F32, tag="t2t")
                        for q in range(NPAIR):
                            nc.scalar.activation(
                                out=t2t[:, q], in_=t2p[:, q * NG:(q + 1) * NG],
                                func=AF.Tanh,
                            )
                        t2s = work.tile([128, NPAIR, NG], F32R, tag="t2s")
                        nc.vector.tensor_mul(t2s[:, 0], t2t[:, 0], gbp[:, 0:NG])
                        nc.vector.tensor_mul(t2s[:, 1], t2t[:, 1], gbp[:, NG:2 * NG])

                        # ---- U projection into transient PSUM ----
                        # start=True only on each bank's first chunk (c even):
                        # start resets the whole bank's has_written bits, and
                        # the odd chunk then overwrites-where-clear.
                        upsum = uppool.tile([128, NCH * NG], F32, tag="up")
                        for c in range(NCH):
                            for q in range(NPAIR):
                                nc.tensor.matmul(
                                    upsum[:, c * NG:(c + 1) * NG],
                                    uws[:, l, q, c, :],
                                    t2s[:, q],
                                    start=(q == 0 and c % 2 == 0),
                                    stop=(q == NPAIR - 1),
                                    skip_group_check=True,
                                )

                        # ---- xl_{l+1} = xl_l + x0*(upsum + bias_l) ----
                        # d = x0*(upsum+b) on DVE; the residual add runs as a
                        # software-DGE accumulate DMA (SBUF->SBUF, issued by
                        # the otherwise idle Pool engine, no HBM traffic)
                        if l == 0:
                            dst = xl            # xl_1 = x0*(upsum+1+b0)
                        elif l == L - 1:
                            dst = work.tile([128, NCH, NG], F32R, tag="outs")
                        else:
                            dst = work.tile([128, NCH, NG], F32R, tag="tmp")
                        if with_bias:
                            for c in range(NCH):
                                nc.vector.scalar_tensor_tensor(
                                    out=dst[:, c],
                                    in0=upsum[:, c * NG:(c + 1) * NG],
                                    scalar=bcs[:, l, c:c + 1],
                                    in1=x0s[:, c],
                                    op0=ALU.add, op1=ALU.mult,
                                )
                        else:
                            for c0 in (0, 4):
                                up_sl = upsum[:, c0 * NG:(c0 + 4) * NG]
                                if l == 0:
                                    nc.vector.scalar_tensor_tensor(
                                        out=dst[:, c0:c0 + 4], in0=up_sl,
                                        scalar=1.0, in1=x0s[:, c0:c0 + 4],
                                        op0=ALU.add, op1=ALU.mult,
                                    )
                                else:
                                    nc.vector.tensor_mul(
                                        dst[:, c0:c0 + 4], up_sl,
                                        x0s[:, c0:c0 + 4],
                                    )
                        if l == 1:
                            nc.gpsimd.dma_start(out=xl, in_=dst,
                                                accum_op=ALU.add)
                        elif l == L - 1:
                            nc.gpsimd.dma_start(out=dst, in_=xl,
                                                accum_op=ALU.add)
                            nc.sync.dma_start(out=outT.ap()[:, g], in_=dst)

    nc.compile()
    return nc


_PROG_CACHE: dict[bool, object] = {}


def _get_program(with_bias: bool):
    if with_bias not in _PROG_CACHE:
        _PROG_CACHE[with_bias] = build_program(with_bias)
    return _PROG_CACHE[with_bias]


def prepare_inputs(x, U, V, C, bias, gate_w):
    """Host-side prep: pack weights into SBUF layouts, shard x. Returns in_maps."""
    x = np.asarray(x, dtype=np.float32)
    U = np.asarray(U, dtype=np.float32)
    V = np.asarray(V, dtype=np.float32)
    C = np.asarray(C, dtype=np.float32)
    bias = np.asarray(bias, dtype=np.float32)
    gate_w = np.asarray(gate_w, dtype=np.float32)

    # V[l,e] is [R,F]; lhsT needs [F,R] chunks, experts packed in pairs.
    # Final layout [p, l, q, c, m]: element (l, q, f=c*128+p, m).
    vt = V.transpose(0, 1, 3, 2)                     # [L,E,F,R]
    vwm = np.stack(
        [np.concatenate([vt[:, 2 * q], vt[:, 2 * q + 1]], axis=-1)
         for q in range(NPAIR)], axis=1,
    )                                                # [L,NPAIR,F,128]
    vwh = np.ascontiguousarray(
        vwm.reshape(L, NPAIR, NCH, 128, 128).transpose(3, 0, 1, 2, 4)
    )                                                # [128,L,NPAIR,NCH,128]

    # C[l,e] is [S,R]; lhsT needs [R,S]; block-diag per pair.
    ct = C.transpose(0, 1, 3, 2)                     # [L,E,R,S]
    cwm = np.zeros((L, NPAIR, 128, 128), dtype=np.float32)
    for q in range(NPAIR):
        cwm[:, q, :R, :R] = ct[:, 2 * q]
        cwm[:, q, R:, R:] = ct[:, 2 * q + 1]
    cwh = np.ascontiguousarray(cwm.transpose(2, 0, 1, 3))   # [128,L,NPAIR,128]

    # U[l,e] is [F,S]; lhsT needs [S,F] stacked per pair.
    ut = U.transpose(0, 1, 3, 2)                     # [L,E,S,F]
    uwm = np.stack(
        [np.concatenate([ut[:, 2 * q], ut[:, 2 * q + 1]], axis=1)
         for q in range(NPAIR)], axis=1,
    )                                                # [L,NPAIR,128,F]
    uwh = np.ascontiguousarray(
        uwm.reshape(L, NPAIR, 128, NCH, 128).transpose(2, 0, 1, 3, 4)
    )                                                # [128,L,NPAIR,NCH,128]

    gwh = np.ascontiguousarray(
        gate_w.T.reshape(NCH, 128, E).transpose(1, 0, 2)
    )                                                # [128,NCH,E]

    # broadcast patterns: p4[e,q,m] = 1 if expert e feeds rows m of pair q
    p4h = np.zeros((4, NPAIR, 128), dtype=np.float32)
    for q in range(NPAIR):
        p4h[2 * q, q, :R] = 1.0
        p4h[2 * q + 1, q, R:] = 1.0

    bias2 = bias[:, :, 0] if bias.ndim == 3 else bias
    with_bias = bool(np.any(bias2))
    # per-layer increments for the running multiplier state a (+1 at l=0)
    binc = bias2.copy()
    binc[0] += 1.0
    bch = np.ascontiguousarray(
        binc.reshape(L, NCH, 128).transpose(2, 0, 1).astype(np.float32)
    )                                                # [128,L,NCH]

    shared = {
        "vw": vwh, "cw": cwh, "uw": uwh, "gw": gwh, "p4": p4h, "bcb": bch,
        "ones": np.ones((4, 512), dtype=np.float32),
    }
    in_maps = []
    for k in range(N_CORES):
        xc = np.ascontiguousarray(x[k * BC:(k + 1) * BC].T)  # [F,BC]
        xck = xc.reshape(NCH, 128, GROUPS, NG).transpose(1, 2, 0, 3)
        m = dict(shared)
        m["xh"] = np.ascontiguousarray(xck)          # [128,GROUPS,NCH,NG]
        in_maps.append(m)
    return in_maps, with_bias


def unpack_output(res) -> np.ndarray:
    """Assemble the [B,F] float32 output from per-core outT tensors."""
    out = np.empty((B, F), dtype=np.float32)
    for k in range(N_CORES):
        o = np.asarray(res.results[k]["outT"])       # [128,GROUPS,NCH,NG]
        if o.dtype != np.float32:
            o = o.view(np.float32)                   # f32r == fp32 bits
        oc = o.transpose(2, 0, 1, 3).reshape(F, BC)  # [F,BC]
        out[k * BC:(k + 1) * BC] = oc.T
    return out


def run(in_maps, with_bias, **kw):
    nc = _get_program(with_bias)
    return run_bass_kernel_spmd(nc, in_maps, list(range(N_CORES)), **kw)


def kernel(x, U, V, C, bias, gate_w):
    in_maps, with_bias = prepare_inputs(x, U, V, C, bias, gate_w)
    res = run(in_maps, with_bias)
    return unpack_output(res)
